# revision 59
# baseline (speedup 1.0000x reference)
"""TRN2 Bass kernel for nn_LocalSelfAttn (LN -> packed QKV -> banded attention
(window +-16) -> out-proj -> residual), sharded 8-way over (B, T).

Sharding: 8 cores x 1024 tokens (batch b = cores 4b..4b+3). Each core gets a
halo-padded strip of 1152 tokens (64 halo each side, zero-padded at batch
edges) and computes its 1024 output tokens independently -- no collectives.

v2 design (per core):
  A. LN via bn_stats on bf16 x; h written fp8 by ACT; h^T via PE transposes.
  B. QKV projections as fp8 DoubleRow matmuls (2x128 contraction per instr,
     0.5 cyc/col). Q^T/K^T drained to bf16 with descales; V token-major
     (carries a 16x scale) restrided to per-head [tok,65] slots whose last
     column is 1.0 so PV emits softmax denominators for free.
  C. Attention in 96-query blocks (11 per core): window = exactly 128 keys.
     S^T computed k-major ([128 keys, 8h, 96q]) so no P transposes are
     needed; exp on ACT; band mask multiplied post-exp on GPSIMD; PV with
     em stationary gives o[96, 8, 65] (col 64 = sum). Normalization via
     si broadcast + multiply during the PSUM drain; o^T via PE transposes.
  D. Out-proj as fp8 DoubleRow with o^T stationary per 128-token group;
     residual x added in-PSUM via a 256*I matmul (yp carries 256x scale);
     single tensor_scalar drain (x 1/256) then DMA.

Scales: wq x64 (incl 1/sqrt(dh)), wk x16, wv x16, wo x16; drains descale
Q by 1/64, K by 1/16; v stays 16x; o_norm = 16x true (fp8-friendly);
yp = 256x true + 256x resid; final drain x 1/256.
"""

import sys

for _p in ("/opt/trn_rl_repo",):
    if _p not in sys.path:
        sys.path.insert(0, _p)

import numpy as np
import ml_dtypes

import concourse.bass as bass
import concourse.tile as tile
from concourse import bacc, mybir
from concourse.bass import ts
from concourse.bass_utils import run_bass_kernel_spmd
from concourse.masks import make_identity

F32 = mybir.dt.float32
BF16 = mybir.dt.bfloat16
FP8 = mybir.dt.float8e4
AF = mybir.ActivationFunctionType
ALU = mybir.AluOpType
DR = mybir.MatmulPerfMode.DoubleRow

B, T, D, H, BAND = 2, 4096, 512, 8, 16
DH = D // H            # 64
LN_EPS = 1e-5
N_CORES = 8
PC = 1024              # tokens per core
HALO = 64
ST = PC + 2 * HALO     # strip tokens = 1152
NT = ST // 128         # 9 x-tiles
QB = 96                # queries per attention block
NB = 11                # attention blocks (11*96 = 1056 >= 1024)
WIN = 128              # key window per block (96 + 2*16)
QT = NB * QB           # 1056 q columns computed
Q0 = HALO              # strip offset of first own query
K0 = HALO - BAND       # strip offset of first window (48)

_NC_CACHE = {}
import os
KSTAGE = os.environ.get("KSTAGE", "full")  # A | B | C | full


def build_bass(with_bias):
    nc = bacc.Bacc(None, target_bir_lowering=False)
    xin = nc.declare_dram_parameter("xin", [ST, D], BF16, isOutput=False)
    wq = nc.declare_dram_parameter("wq", [128, 4, D], FP8, isOutput=False)
    wk = nc.declare_dram_parameter("wk", [128, 4, D], FP8, isOutput=False)
    wv = nc.declare_dram_parameter("wv", [128, 4, D], FP8, isOutput=False)
    wo = nc.declare_dram_parameter("wo", [128, 4, D], FP8, isOutput=False)
    # additive band mask (block-invariant): 0 in-band, -28 out (exp(-28) ~ 0)
    bmask = nc.declare_dram_parameter("bmask", [128, 4 * QB], FP8, isOutput=False)
    # per-token softmax-denominator gates (0 outside the global [0,T) range)
    vones = nc.declare_dram_parameter("vones", [128, NT, H], FP8, isOutput=False)
    icon = nc.declare_dram_parameter("icon", [2, 128, 128], BF16, isOutput=False)
    if with_bias:
        # beffqk: per-partition bias for the 8 q/k output chunks (scaled like
        # the drains expect); boutrow: out-proj bias row (x256).
        beffqk = nc.declare_dram_parameter("beffqk", [128, 8], F32, isOutput=False)
        boutrow = nc.declare_dram_parameter("boutrow", [1, D], BF16, isOutput=False)
        onesrow = nc.declare_dram_parameter("onesrow", [1, 128], BF16, isOutput=False)
    yout = nc.declare_dram_parameter("yout", [PC, D], BF16, isOutput=True)

    with tile.TileContext(nc) as tc:
        from contextlib import ExitStack

        with ExitStack() as ctx:
            const = ctx.enter_context(tc.tile_pool(name="const", bufs=1))
            sb = ctx.enter_context(tc.tile_pool(name="sb", bufs=1))
            ln = ctx.enter_context(tc.tile_pool(name="ln", bufs=4))
            at = ctx.enter_context(tc.tile_pool(name="at", bufs=3))

            # ---- constants ----
            wq_sb = const.tile([128, 4, D], FP8)
            nc.sync.dma_start(out=wq_sb, in_=wq[:, :, :])
            wk_sb = const.tile([128, 4, D], FP8)
            nc.sync.dma_start(out=wk_sb, in_=wk[:, :, :])
            wv_sb = const.tile([128, 4, D], FP8)
            nc.sync.dma_start(out=wv_sb, in_=wv[:, :, :])
            wo_sb = const.tile([128, 4, D], FP8)
            nc.sync.dma_start(out=wo_sb, in_=wo[:, :, :])
            bm_sb = const.tile([128, 4, QB], FP8)
            nc.sync.dma_start(out=bm_sb, in_=bmask[:, :])
            ic_sb = const.tile([128, 2, 128], BF16)   # [0]=256*I, [1]=256*shift64
            nc.sync.dma_start(out=ic_sb, in_=icon.rearrange("c p j -> p c j"))
            ident = const.tile([128, 128], BF16)
            make_identity(nc, ident)
            identf8 = const.tile([128, 128], FP8)
            make_identity(nc, identf8)
            eps_sb = const.tile([128, 1], F32)
            nc.vector.memset(eps_sb, LN_EPS)
            if with_bias:
                beff_sb = const.tile([128, 8], F32)
                nc.sync.dma_start(out=beff_sb, in_=beffqk[:, :])
                bo_sb = const.tile([1, D], BF16)
                nc.sync.dma_start(out=bo_sb, in_=boutrow[:, :])
                ones_sb = const.tile([1, 128], BF16)
                nc.sync.dma_start(out=ones_sb, in_=onesrow[:, :])

            # ---- persistent activations ----
            x_sb = sb.tile([128, NT, D], BF16)         # input tiles (also residual)
            nc.sync.dma_start(out=x_sb, in_=xin.rearrange("(t p) d -> p t d", p=128))
            ht_sb = sb.tile([128, 4, ST], FP8)         # h^T: [D(4x128), tok]
            qT_sb = sb.tile([128, 4, QT], BF16)        # q^T: [512(4x128), 1056]
            kT_sb = sb.tile([128, 4, ST], BF16)        # k^T: [512(4x128), 1152]
            v_sb = sb.tile([128, NT, H, DH + 1], FP8)  # v*16 tok-major + gate col
            v_blk = sb.tile([128, NB, H, DH + 1], FP8)  # per-block rotated windows
            oT_sb = sb.tile([128, 4, QT], FP8)         # o^T*16: [512, 1056]

            # denominator gate columns (also for v_blk: copies preserve them)
            nc.sync.dma_start(out=v_sb[:, :, :, DH:DH + 1], in_=vones[:, :, :])

            # ================= Phase A: LN + h^T =================
            with tc.tile_pool(name="psA", bufs=2, space="PSUM") as psA:
                for t in range(NT):
                    stats = ln.tile([128, 6], F32)
                    nc.vector.bn_stats(out=stats, in_=x_sb[:, t, :])
                    mv = ln.tile([128, 2], F32)
                    nc.vector.bn_aggr(out=mv, in_=stats)
                    std = ln.tile([128, 1], F32)
                    nc.scalar.activation(out=std, in_=mv[:, 1:2], func=AF.Sqrt, bias=eps_sb)
                    rstd = ln.tile([128, 1], F32)
                    nc.vector.reciprocal(out=rstd, in_=std)
                    nbias = ln.tile([128, 1], F32)
                    nc.vector.tensor_scalar(
                        out=nbias, in0=mv[:, 0:1], scalar1=rstd, scalar2=-1.0,
                        op0=ALU.mult, op1=ALU.mult)
                    hbf = ln.tile([128, D], BF16, tag="hbf")
                    nc.scalar.activation(out=hbf, in_=x_sb[:, t, :], func=AF.Identity,
                                         bias=nbias, scale=rstd)
                    tp = psA.tile([128, 4, 128], BF16, tag="tr")
                    for ic in range(4):
                        nc.tensor.transpose(tp[:, ic, :], hbf[:, ts(ic, 128)], ident)
                    nc.vector.tensor_copy(ht_sb[:, :, ts(t, 128)], tp)

            if KSTAGE == "A":
                for g in range(8):
                    ysb = at.tile([128, 4, 128], BF16, tag="ysb")
                    nc.vector.tensor_copy(ysb, ht_sb[:, :, ts(g, 128)])
                    nc.sync.dma_start(out=yout[ts(g, 128), :], in_=ysb)

            # ================= Phase B: QKV projections (fp8 DoubleRow) ======
            with tc.tile_pool(name="psB", bufs=3, space="PSUM") as psB:
              if KSTAGE != "A":
                # Q^T: tokens Q0..Q0+QT (1056): chunks 512,512,32
                qchunks = [(0, 512), (512, 512), (1024, 32)]
                for jc in range(4):
                    for s0, sn in qchunks:
                        qp = psB.tile([128, 512], F32, tag="qk")
                        for c in range(2):
                            nc.tensor.matmul(
                                qp[:, 0:sn],
                                lhsT=wq_sb[:, 2 * c:2 * c + 2, ts(jc, 128)],
                                rhs=ht_sb[:, 2 * c:2 * c + 2, Q0 + s0:Q0 + s0 + sn],
                                start=(c == 0), stop=(c == 1), perf_mode=DR)
                        if with_bias:
                            nc.vector.tensor_scalar(
                                out=qT_sb[:, jc, s0:s0 + sn], in0=qp[:, 0:sn],
                                scalar1=1.0 / 64, scalar2=beff_sb[:, jc:jc + 1],
                                op0=ALU.mult, op1=ALU.add)
                        else:
                            nc.vector.tensor_scalar_mul(
                                out=qT_sb[:, jc, s0:s0 + sn], in0=qp[:, 0:sn],
                                scalar1=1.0 / 64)
                # K^T: all 1152 tokens: chunks 512,512,128
                kchunks = [(0, 512), (512, 512), (1024, 128)]
                for jc in range(4):
                    for s0, sn in kchunks:
                        kp = psB.tile([128, 512], F32, tag="qk")
                        for c in range(2):
                            nc.tensor.matmul(
                                kp[:, 0:sn],
                                lhsT=wk_sb[:, 2 * c:2 * c + 2, ts(jc, 128)],
                                rhs=ht_sb[:, 2 * c:2 * c + 2, s0:s0 + sn],
                                start=(c == 0), stop=(c == 1), perf_mode=DR)
                        if with_bias:
                            nc.vector.tensor_scalar(
                                out=kT_sb[:, jc, s0:s0 + sn], in0=kp[:, 0:sn],
                                scalar1=1.0 / 16, scalar2=beff_sb[:, 4 + jc:5 + jc],
                                op0=ALU.mult, op1=ALU.add)
                        else:
                            nc.scalar.mul(
                                kT_sb[:, jc, s0:s0 + sn], kp[:, 0:sn], 1.0 / 16)
                # V token-major (x16), aligned 128-token chunks
                for vt in range(NT):
                    vp = psB.tile([128, 512], F32, tag="qk")
                    for c in range(2):
                        nc.tensor.matmul(
                            vp, lhsT=ht_sb[:, 2 * c:2 * c + 2, ts(vt, 128)],
                            rhs=wv_sb[:, 2 * c:2 * c + 2, :],
                            start=(c == 0), stop=(c == 1), perf_mode=DR)
                    # restride into per-head [tok, 65] slots (col 64 stays 1.0)
                    nc.vector.tensor_copy(
                        v_sb[:, vt, :, 0:DH],
                        vp.rearrange("p (h d) -> p h d", h=H))

            if KSTAGE == "B":
                for g in range(8):
                    ysb = at.tile([128, 4, 128], BF16, tag="ysb")
                    nc.vector.tensor_copy(ysb, qT_sb[:, :, ts(g, 128)])
                    nc.sync.dma_start(out=yout[ts(g, 128), :], in_=ysb)

            # odd-head q/k rows shifted to base partition 0 (matmuls cannot
            # source lhsT/rhs at base partition 64)
            qT2_sb = sb.tile([64, 4, QT], BF16)
            nc.sync.dma_start(out=qT2_sb, in_=qT_sb[64:128, :, :])
            kT2_sb = sb.tile([64, 4, ST], BF16)
            nc.sync.dma_start(out=kT2_sb, in_=kT_sb[64:128, :, :])
            x0s_sb = sb.tile([64, D], BF16)
            nc.sync.dma_start(out=x0s_sb, in_=x_sb[64:128, 0, :])

            # v_blk: window-aligned copies (partition-rotated via SBUF DMA);
            # 128-aligned windows read v_sb directly (see vsrc below)
            def vsrc(b):
                w0 = K0 + QB * b
                c0, r = divmod(w0, 128)
                if r == 0:
                    return v_sb[:, c0]
                return v_blk[:, b]

            # batch rotations by shared r: blocks b, b+4, b+8 share r
            from collections import defaultdict as _dd
            _rgroups = _dd(list)
            for b in range(NB):
                c0, r = divmod(K0 + QB * b, 128)
                if r:
                    _rgroups[r].append((b, c0))
            for r, blist in _rgroups.items():
                bs = [b for b, _ in blist]
                cs = [c for _, c in blist]
                # strided groups: b step 4, c0 step 3
                assert bs == list(range(bs[0], bs[0] + 4 * len(bs), 4))
                assert cs == list(range(cs[0], cs[0] + 3 * len(cs), 3))
                nb_, b0, c0_ = len(bs), bs[0], cs[0]
                be = b0 + 4 * (nb_ - 1) + 1
                ce = c0_ + 3 * (nb_ - 1) + 1
                nc.sync.dma_start(
                    out=v_blk[0:128 - r, b0:be:4],
                    in_=v_sb[r:128, c0_:ce:3])
                nc.sync.dma_start(
                    out=v_blk[128 - r:128, b0:be:4],
                    in_=v_sb[0:r, c0_ + 1:ce + 1:3])

            # ================= Phase C: attention blocks =================
            with (
                tc.tile_pool(name="psS", bufs=2, space="PSUM") as psS,
                tc.tile_pool(name="psO", bufs=1, space="PSUM") as psO,
                tc.tile_pool(name="psT", bufs=1, space="PSUM") as psT,
                tc.tile_pool(name="psY", bufs=1, space="PSUM") as psY,
            ):
              if KSTAGE not in ("A", "B"):
                def outproj_group(g):
                    # output tokens = strip [128g, 128g+128) clipped to own range
                    lo = max(128 * g, Q0)
                    hi = min(128 * g + 128, Q0 + PC)
                    n = hi - lo            # 64 (g=0,8) or 128
                    q0 = lo - Q0           # oT col offset
                    yp = psY.tile([128, D], F32, tag="y")
                    for c in range(2):
                        nc.tensor.matmul(
                            yp[0:n, :],
                            lhsT=oT_sb[:, 2 * c:2 * c + 2, q0:q0 + n],
                            rhs=wo_sb[:, 2 * c:2 * c + 2, :],
                            start=(c == 0), stop=False, perf_mode=DR)
                    # residual: yp += 256 * x  (x tile g, partition range lo..hi)
                    p0 = lo - 128 * g
                    xr = x0s_sb if p0 else x_sb[0:n, g, :]
                    nc.tensor.matmul(
                        yp[0:n, :],
                        lhsT=ic_sb[0:n, 0, 0:n],
                        rhs=xr,
                        start=False, stop=(not with_bias))
                    if with_bias:
                        nc.tensor.matmul(
                            yp[0:n, :], lhsT=ones_sb[:, 0:n], rhs=bo_sb,
                            start=False, stop=True)
                    ysb = at.tile([128, D], BF16, tag="ysb")
                    nc.vector.tensor_scalar_mul(
                        out=ysb[0:n, :], in0=yp[0:n, :], scalar1=1.0 / 256)
                    nc.sync.dma_start(out=yout[lo - Q0:hi - Q0, :], in_=ysb[0:n, :])

                done_g = 0
                for b in range(NB):
                    q0 = QB * b            # qT/oT column offset of this block
                    w0 = K0 + QB * b       # strip offset of window start
                    # --- S^T: [128 keys, 8 heads, 96 queries] ---
                    sp0 = psS.tile([128, 4, QB], F32, tag="s0")
                    sp1 = psS.tile([128, 4, QB], F32, tag="s1")
                    for h in range(H):
                        jc = h // 2
                        kt = kT_sb if h % 2 == 0 else kT2_sb
                        qt = qT_sb if h % 2 == 0 else qT2_sb
                        sp = sp0 if h < 4 else sp1
                        nc.tensor.matmul(
                            sp[:, h % 4, :],
                            lhsT=kt[0:64, jc, w0:w0 + WIN],
                            rhs=qt[0:64, jc, q0:q0 + QB],
                            start=(h % 4 == 0), stop=False)
                    # additive band mask in PSUM: sp += I^T @ madd = madd
                    nc.tensor.matmul(sp0, lhsT=identf8, rhs=bm_sb,
                                     start=False, stop=True)
                    nc.tensor.matmul(sp1, lhsT=identf8, rhs=bm_sb,
                                     start=False, stop=True)
                    # --- exp (ACT) ---
                    em = at.tile([128, H, QB], FP8, tag="em")
                    nc.scalar.activation(out=em[:, 0:4, :], in_=sp0, func=AF.Exp)
                    nc.scalar.activation(out=em[:, 4:8, :], in_=sp1, func=AF.Exp)
                    if KSTAGE in ("C0", "C1"):
                        dmy = at.tile([128, D], BF16, tag="dmy")
                        nc.vector.tensor_copy(
                            dmy, em.rearrange("p h q -> p (h q)")[:, 0:D])
                        nc.sync.dma_start(out=yout[ts(b % 8, 128), :], in_=dmy)
                        continue
                    # --- PV: o[96q, 8h, 65] with col 64 = sum ---
                    op0 = psO.tile([QB, 4, DH + 1], F32, tag="o0")
                    op1 = psO.tile([QB, 4, DH + 1], F32, tag="o1")
                    vb = vsrc(b)
                    for h in range(H):
                        op = op0 if h < 4 else op1
                        nc.tensor.matmul(
                            op[:, h % 4, :],
                            lhsT=em[:, h, :], rhs=vb[:, h, :],
                            start=(h % 4 == 0), stop=(h % 4 == 3))
                    # --- normalize + drain to fp8 ---
                    si = at.tile([QB, H, 1], F32, tag="si")
                    nc.vector.reciprocal(out=si[:, 0:4, :], in_=op0[:, :, DH:DH + 1])
                    nc.vector.reciprocal(out=si[:, 4:8, :], in_=op1[:, :, DH:DH + 1])
                    onrm = at.tile([QB, H, DH], BF16, tag="onrm")
                    nc.vector.tensor_tensor(
                        out=onrm[:, 0:4, :], in0=op0[:, :, 0:DH],
                        in1=si[:, 0:4, :].to_broadcast([QB, 4, DH]), op=ALU.mult)
                    nc.vector.tensor_tensor(
                        out=onrm[:, 4:8, :], in0=op1[:, :, 0:DH],
                        in1=si[:, 4:8, :].to_broadcast([QB, 4, DH]), op=ALU.mult)
                    if KSTAGE == "C2":
                        dmy = at.tile([128, D], BF16, tag="dmy")
                        nc.vector.tensor_copy(
                            dmy[0:QB, :], onrm.rearrange("q h d -> q (h d)"))
                        nc.sync.dma_start(out=yout[ts(b % 8, 128)][0:QB, :],
                                          in_=dmy[0:QB, :])
                        continue
                    # --- o^T via PE transposes ---
                    otp = psT.tile([128, 4, QB], BF16, tag="ot")
                    for ic in range(4):
                        nc.tensor.transpose(
                            otp[:, ic, :],
                            onrm.rearrange("q h d -> q (h d)")[:, ts(ic, 128)],
                            ident[0:QB, 0:QB])
                    nc.vector.tensor_copy(oT_sb[:, :, q0:q0 + QB], otp)
                    if KSTAGE == "C3":
                        dmy = at.tile([128, D], BF16, tag="dmy")
                        nc.vector.tensor_copy(
                            dmy[:, 0:QB], oT_sb[:, 0, q0:q0 + QB])
                        nc.sync.dma_start(out=yout[ts(b % 8, 128)][:, 0:QB],
                                          in_=dmy[:, 0:QB])
                        continue
                    # --- issue any out-proj groups now enabled ---
                    # group g needs oT cols up to 128g+64 -> blocks through
                    # ceil((128g+64)/96)-1
                    while done_g <= 8 and 128 * done_g + 64 <= QB * (b + 1):
                        outproj_group(done_g)
                        done_g += 1
                while KSTAGE == "full" and done_g <= 8:
                    outproj_group(done_g)
                    done_g += 1
    nc.finalize()
    return nc


def make_in_maps(x, ln_g, ln_b, w_in, b_in, w_out, b_out):
    x = np.asarray(x, np.float32)
    ln_g = np.asarray(ln_g, np.float32)
    ln_b = np.asarray(ln_b, np.float32)
    w_in = np.asarray(w_in, np.float32)
    b_in = np.asarray(b_in, np.float32)
    w_out = np.asarray(w_out, np.float32)
    b_out = np.asarray(b_out, np.float32)

    # fold LN affine into the packed projection; 1/sqrt(dh) into Q
    w_eff = w_in * ln_g[None, :]
    b_eff = b_in + w_in @ ln_b
    sc = np.float32(1.0 / np.sqrt(DH))
    bout_eff = b_out + w_out @ b_eff[2 * D:]   # V bias passes through softmax

    f8 = ml_dtypes.float8_e4m3
    bf = ml_dtypes.bfloat16

    def wtile(w, scale):
        # [512 out, 512 in] -> lhsT layout [128 in-part, 4 in-chunk, 512 out]
        wt = (w.T * scale).reshape(4, 128, D).transpose(1, 0, 2)
        return np.ascontiguousarray(wt).astype(f8)

    wq = wtile(w_eff[0:D] * sc, 64.0)
    wk = wtile(w_eff[D:2 * D], 16.0)
    wv = wtile(w_eff[2 * D:], 16.0)
    # out-proj rhs needs [hd-part, hd-chunk, 512 out]: wo[p, c, i] = w_out[i, 128c+p]
    wo = (w_out.T * 16.0).reshape(4, 128, D).transpose(1, 0, 2)
    wo = np.ascontiguousarray(wo).astype(f8)

    with_bias = bool(np.any(b_eff != 0.0) or np.any(bout_eff != 0.0))

    # icon: [0] = 256*I128, [1] = 256*shift (row 64+i, col i)
    icon = np.zeros((2, 128, 128), np.float32)
    icon[0] = 256.0 * np.eye(128)
    icon[1, 64:128, 0:64] = 256.0 * np.eye(64)
    icon = icon.astype(bf)

    # additive band mask (block-invariant), replicated for 4 head slots
    kk = np.arange(WIN)[:, None]
    qq = np.arange(QB)[None, :]
    band = (kk - qq >= 0) & (kk - qq <= 2 * BAND)
    bm = np.where(band, 0.0, -28.0).astype(np.float32)
    bm = np.repeat(bm[:, None, :], 4, axis=1).reshape(WIN, 4 * QB).astype(f8)

    in_maps = []
    for core in range(N_CORES):
        batch = core // 4
        t0 = (core % 4) * PC
        lo = t0 - HALO
        s0, s1 = max(lo, 0), min(t0 + PC + HALO, T)
        xloc = np.zeros((ST, D), np.float32)
        xloc[s0 - lo:s1 - lo] = x[batch, s0:s1]

        # denominator gates: 1.0 for strip tokens inside the global range;
        # layout [p, chunk, h] matching v_sb partitions
        gt = lo + np.arange(ST)
        vo = ((gt >= 0) & (gt < T)).astype(np.float32)
        vo = np.repeat(vo[:, None], H, axis=1).reshape(NT, 128, H)
        vo = np.ascontiguousarray(vo.transpose(1, 0, 2)).astype(f8)

        m = dict(xin=xloc.astype(bf), wq=wq, wk=wk, wv=wv, wo=wo,
                 bmask=bm, vones=vo, icon=icon)
        if with_bias:
            beffqk = np.concatenate([
                (b_eff[0:D] * sc / 1.0).reshape(4, 128),
                b_eff[D:2 * D].reshape(4, 128)]).T
            m["beffqk"] = np.ascontiguousarray(beffqk).astype(np.float32)
            m["boutrow"] = (bout_eff * 256.0).reshape(1, D).astype(bf)
            m["onesrow"] = np.ones((1, 128), bf)
        in_maps.append(m)
    return in_maps, with_bias


def kernel_run(inputs, trace=False, trace_kwargs=None):
    in_maps, with_bias = make_in_maps(**inputs)
    key = bool(with_bias)
    if key not in _NC_CACHE:
        _NC_CACHE[key] = build_bass(with_bias)
    nc = _NC_CACHE[key]
    kw = {}
    if trace:
        kw = dict(trace=True, trace_cores=[0], **(trace_kwargs or {}))
    res = run_bass_kernel_spmd(nc, in_maps, list(range(N_CORES)), **kw)
    y = np.stack([np.asarray(res.results[c]["yout"], np.float32)
                  for c in range(N_CORES)])
    out = y.reshape(B, T, D)
    return out, res


def kernel(**inputs):
    out, _ = kernel_run(inputs, trace=False)
    return out


if __name__ == "__main__":
    rng = np.random.default_rng(0)
    ins = dict(
        x=rng.standard_normal((B, T, D)).astype(np.float32),
        ln_g=np.ones(D, np.float32), ln_b=np.zeros(D, np.float32),
        w_in=(rng.standard_normal((3 * D, D)) * 0.02).astype(np.float32),
        b_in=np.zeros(3 * D, np.float32),
        w_out=(rng.standard_normal((D, D)) * 0.02).astype(np.float32),
        b_out=np.zeros(D, np.float32))
    out = kernel(**ins)
    print("ran:", out.shape, out.dtype)
